# revision 1
# baseline (speedup 1.0000x reference)
# Trainium2 Bass kernel for nn_ComplexLambdaLayer (complex lambda attention layer).
# Sharding: data-parallel over batch b (16) across 8 cores (2 per core).
# The positional-lambda contraction lam_p[b,n,k,v] = sum_m R_k[n,m] V[b,v,m] uses
# the block-Toeplitz structure of R (R[n,m] = emb[pos_m - pos_n + 31]): only 15
# distinct 128x128 blocks per k exist (host-expanded fp16 table M_all), so the
# 1024x1024 matmul becomes 8x8 chunk-matmuls with 15 stationary weights.
# lam_c is folded into the same PSUM chain via an indicator-row matmul.
# Yp = sum_k q*Lam uses a block-diagonal q lhsT (8 n-positions x 16 k = K128).
import numpy as np
from contextlib import ExitStack

import bass_rust
import concourse.bacc as bacc
import concourse.tile as tile
from concourse import mybir
from concourse.bass_utils import run_bass_kernel_spmd

F32 = mybir.dt.float32
F32R = mybir.dt.float32r
F16 = mybir.dt.float16

NCORES = 8
B = 16
BL = 2          # batches per core
DIM = 256
KD = 16         # DIM_K
HEADS = 8
VD = 32         # DIM_V
N2 = 1024
EPS = 1e-5
NSTAT = float(B * N2)

_CACHE = {}


def _build_host_consts(inp):
    # --- M_all: lhsT[(m-chunk),(n-chunk)] = R[n,m] = emb[pos_m - pos_n + 31]
    # M[k, dp+7][ap*32+jp, a*32+j] = emb[4dp + ap - a + 31, jp - j + 31, k, 0]
    er, ei = inp['emb_re'], inp['emb_im']
    a = np.arange(4); j = np.arange(32); dp = np.arange(-7, 8)
    r0 = (4 * dp[:, None, None, None, None] + a[None, :, None, None, None]
          - a[None, None, None, :, None] + 31)
    r1 = j[None, None, :, None, None] - j[None, None, None, None, :] + 31
    r0 = np.broadcast_to(r0, (15, 4, 32, 4, 32))
    r1 = np.broadcast_to(r1, (15, 4, 32, 4, 32))
    Mr = np.moveaxis(er[r0, r1, :, 0], -1, 0).reshape(16, 15, 128, 128)
    Mi = np.moveaxis(ei[r0, r1, :, 0], -1, 0).reshape(16, 15, 128, 128)
    # mk layout: [k][p 128][(ri,d,c) 3840] fp16
    mk = np.empty((16, 128, 2 * 15 * 128), np.float16)
    mk[:, :, :15 * 128] = Mr.transpose(0, 2, 1, 3).reshape(16, 128, 15 * 128)
    mk[:, :, 15 * 128:] = Mi.transpose(0, 2, 1, 3).reshape(16, 128, 15 * 128)

    # --- W stacks: proj out_r chain uses [Wr; -Wi], out_i chain [Wi; Wr]
    # o-order: 0-127 q(h,k), 128-143 k-proj, 144-175 v-proj; lhsT layout [c,o]
    # q channel order k*8+h (so PE-transposed q has h contiguous for qdiag DMA)
    po = np.arange(128)
    qperm = (po % 8) * 16 + po // 8          # new p -> old index h*16+k
    Wr = np.concatenate([inp['wq_re'][qperm], inp['wk_re'], inp['wv_re']], 0).T
    Wi = np.concatenate([inp['wq_im'][qperm], inp['wk_im'], inp['wv_im']], 0).T
    # cols: 0-127 q, 128-143 k, 144-159 pad, 160-191 v (32-aligned psum bases)
    Wr = np.concatenate([Wr[:, :144], np.zeros((256, 16), np.float32), Wr[:, 144:]], 1)
    Wi = np.concatenate([Wi[:, :144], np.zeros((256, 16), np.float32), Wi[:, 144:]], 1)
    wstk = np.empty((2, 2, 2, 128, 192), np.float32)  # [outri][cc][ri_in]
    for cc in range(2):
        sl = slice(cc * 128, (cc + 1) * 128)
        wstk[0, cc, 0] = Wr[sl]; wstk[0, cc, 1] = -Wi[sl]
        wstk[1, cc, 0] = Wi[sl]; wstk[1, cc, 1] = Wr[sl]

    # --- eyerow for lam_c fold: [16, 16*128] fp16, eyerow[kk, k*128+c] = (kk==k)
    eyerow = np.zeros((16, 16 * 128), np.float16)
    for k in range(16):
        eyerow[k, k * 128:(k + 1) * 128] = 1.0

    ident = np.eye(128, dtype=np.float32)
    ident16 = np.eye(128, dtype=np.float16)

    # --- BN params tile [128, 8]: q Ar-src cols 0-3 (qs_r qs_i qb_r qb_i),
    # v on rows 0-31 cols 4-7
    bnp = np.zeros((128, 8), np.float32)
    bnp[:, 0] = inp['qs_re'][qperm]; bnp[:, 1] = inp['qs_im'][qperm]
    bnp[:, 2] = inp['qb_re'][qperm]; bnp[:, 3] = inp['qb_im'][qperm]
    bnp[:32, 4] = inp['vs_re']; bnp[:32, 5] = inp['vs_im']
    bnp[:32, 6] = inp['vb_re']; bnp[:32, 7] = inp['vb_im']
    return mk, wstk.reshape(8, 128, 192), eyerow, ident, ident16, bnp


def _build_nc():
    nc = bacc.Bacc("TRN2", target_bir_lowering=False, num_devices=NCORES)
    xf_d = nc.declare_dram_parameter("xf", [8, 128, N2], F32R, isOutput=False)
    w_d = nc.declare_dram_parameter("wstk", [8, 128, 192], F32R, isOutput=False)
    mk_d = nc.declare_dram_parameter("mk", [16, 128, 3840], F16, isOutput=False)
    eye_d = nc.declare_dram_parameter("eyerow", [16, 2048], F16, isOutput=False)
    id_d = nc.declare_dram_parameter("ident", [128, 128], F32, isOutput=False)
    id16_d = nc.declare_dram_parameter("ident16", [128, 128], F16, isOutput=False)
    bnp_d = nc.declare_dram_parameter("bnp", [128, 8], F32, isOutput=False)
    yr_d = nc.declare_dram_parameter("yr", [BL, 64, 4096], F32, isOutput=True)
    yi_d = nc.declare_dram_parameter("yi", [BL, 64, 4096], F32, isOutput=True)
    arin = nc.dram_tensor("arin", [128, 10], F32)
    lam2 = nc.dram_tensor("lam2", [4, 128, 1024], F16)
    qd2 = nc.dram_tensor("qd2", [8, 128, 1024], F16)
    arout = nc.dram_tensor("arout", [128, 10], F32, addr_space="Shared")

    with tile.TileContext(nc) as tc, ExitStack() as ctx:
        per = ctx.enter_context(tc.tile_pool(name="per", bufs=1))   # persistent
        tmp = ctx.enter_context(tc.tile_pool(name="tmp", bufs=2))   # scratch

        wt = [per.tile([128, 192], F32R, tag=f"w{i}", name=f"w{i}") for i in range(8)]
        for i in range(8):
            nc.sync.dma_start(wt[i][:], w_d[i])
        eye = per.tile([16, 2048], F16, tag="eye", name="eye")
        nc.sync.dma_start(eye[:], eye_d[:])
        ident = per.tile([128, 128], F32, tag="ident", name="ident")
        nc.sync.dma_start(ident[:], id_d[:])
        ident16 = per.tile([128, 128], F16, tag="ident16", name="ident16")
        nc.sync.dma_start(ident16[:], id16_d[:])
        bnp = per.tile([128, 8], F32, tag="bnp", name="bnp")
        nc.sync.dma_start(bnp[:], bnp_d[:])

        q_sb = [[per.tile([128, N2], F32, tag=f"q{b}{ri}", name=f"q{b}{ri}") for ri in range(2)]
                for b in range(BL)]
        k_sb = [[per.tile([16, N2], F32, tag=f"k{b}{ri}", name=f"k{b}{ri}") for ri in range(2)]
                for b in range(BL)]
        v_sb = [[per.tile([32, N2], F32, tag=f"v{b}{ri}", name=f"v{b}{ri}") for ri in range(2)]
                for b in range(BL)]

        # ---------------- projections (f32r, N=512) ----------------
        with tc.tile_pool(name="xfp", bufs=1) as xfp, \
             tc.tile_pool(name="pj", bufs=4, space="PSUM") as pj:
            xft = [xfp.tile([128, N2], F32R, tag=f"xf{i}", name=f"xf{i}") for i in range(8)]
            for i in range(8):
                nc.sync.dma_start(xft[i][:], xf_d[i])
            for b in range(BL):
                for ri in range(2):
                    for nch in range(2):
                        pq = pj.tile([128, 512], F32, tag="pq", name="pq")
                        pkv = pj.tile([64, 512], F32, tag="pkv", name="pkv")
                        first = True
                        for cc in range(2):
                            for rin in range(2):
                                lhs = wt[ri * 4 + cc * 2 + rin]
                                rhs = xft[b * 4 + rin * 2 + cc][:, nch * 512:(nch + 1) * 512]
                                nc.tensor.matmul(pq[:], lhs[:, 0:128], rhs,
                                                 start=first, stop=(cc == 1 and rin == 1))
                                nc.tensor.matmul(pkv[:], lhs[:, 128:192], rhs,
                                                 start=first, stop=(cc == 1 and rin == 1))
                                first = False
                        sl = slice(nch * 512, (nch + 1) * 512)
                        nc.vector.tensor_copy(q_sb[b][ri][:, sl], pq[:])
                        nc.vector.tensor_copy(k_sb[b][ri][:, sl], pkv[0:16, :])
                        nc.vector.tensor_copy(v_sb[b][ri][:, sl], pkv[32:64, :])

        # ---------------- BN stats + AllReduce ----------------
        stats = per.tile([128, 10], F32, tag="stats", name="stats")
        st_sc = [tmp.tile([128, 1], F32, tag=f"sc{i}", name=f"sc{i}") for i in range(4)]
        scrap = [tmp.tile([128, N2], F32, tag=f"scr{i}", name=f"scr{i}") for i in range(2)]

        def stat5(planes, rows, coff):
            # planes[b][ri] with `rows` partitions -> stats[:, coff:coff+5]
            for s_i, expr in enumerate(["r", "i", "rr", "ii", "ri"]):
                parts = []
                for b in range(BL):
                    t = st_sc[b]
                    pr, pi = planes[b][0][0:rows, :], planes[b][1][0:rows, :]
                    if expr == "r":
                        nc.vector.tensor_reduce(t[0:rows, :], pr, mybir.AxisListType.X,
                                                mybir.AluOpType.add)
                    elif expr == "i":
                        nc.vector.tensor_reduce(t[0:rows, :], pi, mybir.AxisListType.X,
                                                mybir.AluOpType.add)
                    else:
                        a_, b_ = (pr, pr) if expr == "rr" else (pi, pi) if expr == "ii" else (pr, pi)
                        nc.vector.tensor_mul(scrap[b][0:rows, :], a_, b_)
                        nc.vector.tensor_reduce(t[0:rows, :], scrap[b][0:rows, :],
                                                mybir.AxisListType.X, mybir.AluOpType.add)
                    parts.append(t)
                nc.vector.tensor_add(stats[0:rows, coff + s_i:coff + s_i + 1],
                                     parts[0][0:rows, :], parts[1][0:rows, :])

        stat5(q_sb, 128, 0)
        stat5(v_sb, 32, 5)
        nc.sync.dma_start(arin[:], stats[:])
        nc.gpsimd.collective_compute(
            "AllReduce", mybir.AluOpType.add,
            replica_groups=[list(range(NCORES))],
            ins=[arin[:]], outs=[arout[:]])
        ar = per.tile([128, 10], F32, tag="ar", name="ar")
        nc.sync.dma_start(ar[:], arout[:])

        # ---------------- BN coefficients ----------------
        coef = per.tile([128, 8], F32, tag="coef", name="coef")   # q: Ar Ai Br Bi cols0-3; v cols4-7
        ct = [tmp.tile([128, 1], F32, tag=f"ct{i}", name=f"ct{i}") for i in range(8)]

        def bn_coef(rows, soff, poff, coff):
            r_ = slice(0, rows)
            mr, mi, t0, t1, t2, t3, sr, si = (c[r_, :] for c in ct)
            A = lambda c: ar[r_, soff + c:soff + c + 1]
            P = lambda c: bnp[r_, poff + c:poff + c + 1]
            C = lambda c: coef[r_, coff + c:coff + c + 1]
            inv = 1.0 / NSTAT
            nc.vector.tensor_scalar_mul(mr, A(0), inv)
            nc.vector.tensor_scalar_mul(mi, A(1), inv)
            # zr = (err - eii)/N - mr^2 + mi^2 + EPS
            nc.vector.tensor_sub(t0, A(2), A(3))
            nc.vector.tensor_scalar_mul(t0, t0, inv)
            nc.vector.tensor_mul(t1, mr, mr)
            nc.vector.tensor_sub(t0, t0, t1)
            nc.vector.tensor_mul(t1, mi, mi)
            nc.vector.tensor_add(t0, t0, t1)
            nc.vector.tensor_scalar_add(t0, t0, EPS)          # t0 = zr
            # zi = 2*(eri/N - mr*mi)
            nc.vector.tensor_scalar_mul(t1, A(4), inv)
            nc.vector.tensor_mul(t2, mr, mi)
            nc.vector.tensor_sub(t1, t1, t2)
            nc.vector.tensor_scalar_mul(t1, t1, 2.0)          # t1 = zi
            # mag = sqrt(zr^2+zi^2)
            nc.vector.tensor_mul(t2, t0, t0)
            nc.vector.tensor_mul(t3, t1, t1)
            nc.vector.tensor_add(t2, t2, t3)
            nc.scalar.sqrt(t2, t2)                            # t2 = mag
            # sr = sqrt((mag+zr)/2); si = zi/(2 sr)
            nc.vector.tensor_add(t3, t2, t0)
            nc.scalar.activation(sr, t3, mybir.ActivationFunctionType.Sqrt, scale=0.5)
            nc.vector.reciprocal(t3, sr)
            nc.vector.tensor_mul(si, t1, t3)
            nc.vector.tensor_scalar_mul(si, si, 0.5)          # si = zi/(2 sr)
            nc.vector.reciprocal(t3, t2)                      # t3 = 1/mag
            # Ar = (qsr*sr + qsi*si)/mag ; Ai = (qsi*sr - qsr*si)/mag
            nc.vector.tensor_mul(t0, P(0), sr)
            nc.vector.tensor_mul(t1, P(1), si)
            nc.vector.tensor_add(t0, t0, t1)
            nc.vector.tensor_mul(C(0), t0, t3)
            nc.vector.tensor_mul(t0, P(1), sr)
            nc.vector.tensor_mul(t1, P(0), si)
            nc.vector.tensor_sub(t0, t0, t1)
            nc.vector.tensor_mul(C(1), t0, t3)
            # Br = qbr - Ar*mr + Ai*mi ; Bi = qbi - Ar*mi - Ai*mr
            nc.vector.tensor_mul(t0, C(0), mr)
            nc.vector.tensor_sub(t0, P(2), t0)
            nc.vector.tensor_mul(t1, C(1), mi)
            nc.vector.tensor_add(C(2), t0, t1)
            nc.vector.tensor_mul(t0, C(0), mi)
            nc.vector.tensor_sub(t0, P(3), t0)
            nc.vector.tensor_mul(t1, C(1), mr)
            nc.vector.tensor_sub(C(3), t0, t1)

        bn_coef(128, 0, 0, 0)
        bn_coef(32, 5, 4, 4)

        # ---------------- apply BN -> q16 planes, vbn planes ----------------
        q16 = [[per.tile([128, N2], F16, tag=f"q16{b}{ri}", name=f"q16{b}{ri}") for ri in range(2)]
               for b in range(BL)]
        vbn = [[per.tile([32, N2], F32, tag=f"vbn{b}{ri}", name=f"vbn{b}{ri}") for ri in range(2)]
               for b in range(BL)]

        def bn_apply(src, dst, rows, coff):
            r_ = slice(0, rows)
            C = lambda c: coef[r_, coff + c:coff + c + 1]
            for b in range(BL):
                pr, pi = src[b][0][r_, :], src[b][1][r_, :]
                t0, t1 = scrap[0][r_, :], scrap[1][r_, :]
                nc.vector.tensor_scalar(t0, pr, C(0), C(2),
                                        mybir.AluOpType.mult, mybir.AluOpType.add)
                nc.vector.tensor_scalar_mul(t1, pi, C(1))
                nc.vector.tensor_sub(dst[b][0][r_, :], t0, t1)
                nc.vector.tensor_scalar(t0, pi, C(0), C(3),
                                        mybir.AluOpType.mult, mybir.AluOpType.add)
                nc.vector.tensor_scalar_mul(t1, pr, C(1))
                nc.vector.tensor_add(dst[b][1][r_, :], t0, t1)

        bn_apply(q_sb, q16, 128, 0)
        bn_apply(v_sb, vbn, 32, 4)

        # ---------------- softmax(|k|) + ksmT ----------------
        ksmT = [per.tile([128, 128], F16, tag=f"ksmT{b}", name=f"ksmT{b}") for b in range(BL)]
        qT = [[per.tile([128, 1024], F16, tag=f"qT{b}{ri}", name=f"qT{b}{ri}")
               for ri in range(2)] for b in range(BL)]
        V_rhs = per.tile([128, 1024], F16, tag="vrhs", name="vrhs")
        with tc.tile_pool(name="tp", bufs=2, space="PSUM") as tpp:
            for b in range(BL):
                kr, ki = k_sb[b][0], k_sb[b][1]
                ka = scrap[0][0:16, :]
                t1 = scrap[1][0:16, :]
                nc.vector.tensor_mul(ka, kr, kr)
                nc.vector.tensor_mul(t1, ki, ki)
                nc.vector.tensor_add(ka, ka, t1)
                nc.scalar.sqrt(ka, ka)
                mx = st_sc[0][0:16, :]
                nc.vector.tensor_reduce(mx, ka, mybir.AxisListType.X, mybir.AluOpType.max)
                nc.vector.tensor_scalar(ka, ka, mx, None, mybir.AluOpType.subtract)
                sm = st_sc[1][0:16, :]
                nc.scalar.activation(ka, ka, mybir.ActivationFunctionType.Exp,
                                     accum_out=sm)
                rc = st_sc[2][0:16, :]
                nc.vector.reciprocal(rc, sm)
                nc.vector.tensor_scalar(ka, ka, rc, None, mybir.AluOpType.mult)
                for ch in range(8):
                    pt = tpp.tile([128, 16], F32, tag="pt", name="pt")
                    nc.tensor.transpose(pt[:], ka[:, ch * 128:(ch + 1) * 128],
                                        ident[0:16, 0:16])
                    nc.vector.tensor_copy(ksmT[b][:, ch * 16:(ch + 1) * 16], pt[:])
                for ri in range(2):
                    for nb in range(8):
                        pqz = tpp.tile([128, 128], F16, tag="pqz", name="pqz")
                        nc.tensor.transpose(pqz[:], q16[b][ri][:, nb * 128:(nb + 1) * 128],
                                            ident16[:])
                        nc.vector.tensor_copy(qT[b][ri][:, nb * 128:(nb + 1) * 128], pqz[:])
                # V_rhs[(m),(ri,b,v)] from vbn via PE transpose
                for ri in range(2):
                    for ch in range(8):
                        pv = tpp.tile([128, 32], F32, tag="pv", name="pv")
                        nc.tensor.transpose(pv[:], vbn[b][ri][:, ch * 128:(ch + 1) * 128],
                                            ident[0:32, 0:32])
                        nc.vector.tensor_copy(
                            V_rhs[:, ch * 128 + ri * 64 + b * 32:
                                  ch * 128 + ri * 64 + b * 32 + 32], pv[:])

        # ---------------- lam_c ----------------
        lam_sb = per.tile([16, 128], F16, tag="lamc", name="lamc")
        with tc.tile_pool(name="lc", bufs=2, space="PSUM") as lcp:
            for b in range(BL):
                plc = lcp.tile([16, 64], F32, tag="plc", name="plc")
                for ch in range(8):
                    rhs = V_rhs[:, ch * 128 + b * 32:ch * 128 + b * 32 + 1].copy()
                    rhs.ap = bass_rust.VecI64Pair([(1024, 128), (64, 2), (1, 32)])
                    rhs.offset = ch * 128 + b * 32
                    nc.tensor.matmul(plc[:], ksmT[b][:, ch * 16:(ch + 1) * 16], rhs,
                                     start=(ch == 0), stop=(ch == 7))
                for ri in range(2):
                    nc.vector.tensor_copy(
                        lam_sb[:, ri * 64 + b * 32:ri * 64 + b * 32 + 32],
                        plc[:, ri * 32:(ri + 1) * 32])

        # ---------------- lam_p main loop ----------------
        lam_all = [per.tile([128, 2048], F16, tag=f"lam{nb}", name=f"lam{nb}") for nb in range(8)]
        with tc.tile_pool(name="mk", bufs=2) as mkp, \
             tc.tile_pool(name="p2s", bufs=2) as p2sp, \
             tc.tile_pool(name="lp", bufs=2, space="PSUM") as lpp:
            for k in range(16):
                mkt = mkp.tile([128, 3840], F16, tag="mk", name="mk")
                nc.sync.dma_start(mkt[:], mk_d[k])
                for nb in range(8):
                    P1 = lpp.tile([128, 128], F32, tag="P1", name="P1")
                    P2 = lpp.tile([128, 128], F32, tag="P2", name="P2")
                    nc.tensor.matmul(P1[:], eye[:, k * 128:(k + 1) * 128], lam_sb[:],
                                     start=True, stop=False)
                    for bip in range(8):
                        dx = (bip - nb + 7) * 128
                        rhs = V_rhs[:, bip * 128:(bip + 1) * 128]
                        nc.tensor.matmul(P1[:], mkt[:, dx:dx + 128], rhs,
                                         start=False, stop=(bip == 7))
                        nc.tensor.matmul(P2[:], mkt[:, 1920 + dx:1920 + dx + 128], rhs,
                                         start=(bip == 0), stop=(bip == 7))
                    p2s = p2sp.tile([128, 128], F32, tag="p2s", name="p2s")
                    nc.scalar.copy(p2s[:], P2[:])
                    nc.vector.tensor_sub(lam_all[nb][:, k * 128:k * 128 + 64],
                                         P1[:, 0:64], p2s[:, 64:128])
                    nc.vector.tensor_add(lam_all[nb][:, k * 128 + 64:k * 128 + 128],
                                         P1[:, 64:128], p2s[:, 0:64])

        # ---------------- Lam shuffle + qdiag + Yp ----------------
        qd = [[[per.tile([128, 1024], F16, tag=f"qd{b}{ri}{p}", name=f"qd{b}{ri}{p}") for p in range(2)]
               for ri in range(2)] for b in range(BL)]
        for b in range(BL):
            for ri in range(2):
                for p in range(2):
                    nc.vector.memset(qd[b][ri][p][:], 0.0)
                    nc.sync.dma_start(qd2[(b * 2 + ri) * 2 + p], qd[b][ri][p][:])

        with tc.tile_pool(name="lyp", bufs=2) as lypp, \
             tc.tile_pool(name="yo", bufs=1) as yop, \
             tc.tile_pool(name="yp", bufs=2, space="PSUM") as ypp:
            for b in range(BL):
                y_out = [yop.tile([64, 4096], F32, tag=f"yo{ri}", name=f"yo{ri}")
                         for ri in range(2)]
                for nb in range(8):
                    par = nb % 2
                    lamyp = lypp.tile([128, 1024], F16, tag="lamyp", name="lamyp")
                    lidx = b * 2 + par
                    for t in range(16):
                        for ri in range(2):
                            # SBUF[(t,g),(k,ri,b,v)] -> DRAM[(k,g),(t,ri,v)] scatter
                            sap = lam_all[nb][0:1, 0:1].copy()
                            sap.ap = bass_rust.VecI64Pair(
                                [(2048, 8), (128, 16), (1, 32)])
                            sap.offset = t * 8 * 2048 + ri * 64 + b * 32
                            dap = lam2[0][0:1, 0:1].copy()
                            dap.ap = bass_rust.VecI64Pair(
                                [(1024, 8), (8192, 16), (1, 32)])
                            dap.offset = lidx * 131072 + t * 64 + ri * 32
                            nc.scalar.dma_start(dap, sap)
                            # qT[(t,g),(nb;k,h)] -> DRAM[(k,g),(t,g,h)] scatter
                            qap = qT[b][ri][0:1, 0:1].copy()
                            qap.ap = bass_rust.VecI64Pair(
                                [(1024, 8), (8, 16), (1, 8)])
                            qap.offset = t * 8 * 1024 + nb * 128
                            q2ap = qd2[0][0:1, 0:1].copy()
                            q2ap.ap = bass_rust.VecI64Pair(
                                [(1032, 8), (8192, 16), (1, 8)])
                            q2ap.offset = ((b * 2 + ri) * 2 + par) * 131072 + t * 64
                            nc.sync.dma_start(q2ap, qap)
                    nc.scalar.dma_start(lamyp[:], lam2[lidx])
                    for ri in range(2):
                        nc.sync.dma_start(qd[b][ri][par][:],
                                          qd2[(b * 2 + ri) * 2 + par])
                    for t in range(16):
                        P1y = ypp.tile([64, 64], F32, tag="P1y", name="P1y")
                        P2y = ypp.tile([64, 64], F32, tag="P2y", name="P2y")
                        sl = slice(t * 64, (t + 1) * 64)
                        nc.tensor.matmul(P1y[:], qd[b][0][par][:, sl], lamyp[:, sl],
                                         start=True, stop=True)
                        nc.tensor.matmul(P2y[:], qd[b][1][par][:, sl], lamyp[:, sl],
                                         start=True, stop=True)
                        oc = (nb * 16 + t) * 32
                        p2y = lypp.tile([64, 64], F32, tag="p2y", name="p2y")
                        nc.scalar.copy(p2y[:], P2y[:])
                        nc.vector.tensor_sub(y_out[0][:, oc:oc + 32],
                                             P1y[:, 0:32], p2y[:, 32:64])
                        nc.vector.tensor_add(y_out[1][:, oc:oc + 32],
                                             P1y[:, 32:64], p2y[:, 0:32])
                nc.sync.dma_start(yr_d[b], y_out[0][:])
                nc.sync.dma_start(yi_d[b], y_out[1][:])


    nc.compile()
    return nc


def kernel(**inputs):
    inp = {k: np.asarray(v) for k, v in inputs.items()}
    if "nc" not in _CACHE:
        _CACHE["nc"] = _build_nc()
    nc = _CACHE["nc"]
    mk, wstk, eyerow, ident, ident16, bnp = _build_host_consts(inp)

    xr = inp['x_re'].reshape(B, DIM, N2)
    xi = inp['x_im'].reshape(B, DIM, N2)
    in_maps = []
    for c in range(NCORES):
        xf = np.empty((8, 128, N2), np.float32)
        for bl in range(BL):
            b = c * BL + bl
            for ri, xx in ((0, xr), (1, xi)):
                for cc in range(2):
                    xf[bl * 4 + ri * 2 + cc] = xx[b, cc * 128:(cc + 1) * 128, :]
        in_maps.append({"xf": xf, "wstk": wstk, "mk": mk, "eyerow": eyerow,
                        "ident": ident, "ident16": ident16, "bnp": bnp})
    res = run_bass_kernel_spmd(nc, in_maps, list(range(NCORES)))
    out = np.empty((B, 256, N2), np.complex64)
    for c in range(NCORES):
        yr = res.results[c]["yr"]
        yi = res.results[c]["yi"]
        for bl in range(BL):
            yc = (yr[bl] + 1j * yi[bl]).reshape(8, 8, 128, 32)  # (g,h,nt,v)
            out[c * BL + bl] = yc.transpose(1, 3, 2, 0).reshape(256, N2)
    return out.reshape(B, 256, 32, 32)



# revision 50
# speedup vs baseline: 1.5432x; 1.5432x over previous
# Trainium2 Bass kernel for nn_ComplexLambdaLayer (complex lambda attention layer).
# Sharding: data-parallel over batch b (16) across 8 cores (2 per core).
# The positional-lambda contraction lam_p[b,n,k,v] = sum_m R_k[n,m] V[b,v,m] uses
# the block-Toeplitz structure of R (R[n,m] = emb[pos_m - pos_n + 31]): only 15
# distinct 128x128 blocks per k exist (host-expanded fp16 table M_all), so the
# 1024x1024 matmul becomes 8x8 chunk-matmuls with 15 stationary weights.
# lam_c is folded into the same PSUM chain via an indicator-row matmul.
# Yp = sum_k q*Lam uses a block-diagonal q lhsT (8 n-positions x 16 k = K128).
import numpy as np
from contextlib import ExitStack

import bass_rust
import concourse.bacc as bacc
import concourse.tile as tile
from concourse import mybir
from concourse.bass_utils import run_bass_kernel_spmd

F32 = mybir.dt.float32
F32R = mybir.dt.float32r
F16 = mybir.dt.float16

NCORES = 8
B = 16
BL = 2          # batches per core
DIM = 256
KD = 16         # DIM_K
HEADS = 8
VD = 32         # DIM_V
N2 = 1024
EPS = 1e-5
NSTAT = float(B * N2)

_CACHE = {}


def _build_host_consts(inp):
    # --- M_all: lhsT[(m-chunk),(n-chunk)] = R[n,m] = emb[pos_m - pos_n + 31]
    # M[k, dp+7][ap*32+jp, a*32+j] = emb[4dp + ap - a + 31, jp - j + 31, k, 0]
    er, ei = inp['emb_re'], inp['emb_im']
    a = np.arange(4); j = np.arange(32); dp = np.arange(-7, 8)
    r0 = (4 * dp[:, None, None, None, None] + a[None, :, None, None, None]
          - a[None, None, None, :, None] + 31)
    r1 = j[None, None, :, None, None] - j[None, None, None, None, :] + 31
    r0 = np.broadcast_to(r0, (15, 4, 32, 4, 32))
    r1 = np.broadcast_to(r1, (15, 4, 32, 4, 32))
    Mr = np.moveaxis(er[r0, r1, :, 0], -1, 0).reshape(16, 15, 128, 128)
    Mi = np.moveaxis(ei[r0, r1, :, 0], -1, 0).reshape(16, 15, 128, 128)
    # mk layout: [k][p 128][(ri,d,c) 3840] fp16
    mk = np.empty((16, 128, 2 * 15 * 128), np.float16)
    mk[:, :, :15 * 128] = Mr.transpose(0, 2, 1, 3).reshape(16, 128, 15 * 128)
    mk[:, :, 15 * 128:] = Mi.transpose(0, 2, 1, 3).reshape(16, 128, 15 * 128)

    # --- W stacks: proj out_r chain uses [Wr; -Wi], out_i chain [Wi; Wr]
    # o-order: 0-127 q(h,k), 128-143 k-proj, 144-175 v-proj; lhsT layout [c,o]
    # q channel order k*8+h (so PE-transposed q has h contiguous for qdiag DMA)
    po = np.arange(128)
    qperm = (po % 8) * 16 + po // 8          # new p -> old index h*16+k
    Wr = np.concatenate([inp['wq_re'][qperm], inp['wk_re'], inp['wv_re']], 0).T
    Wi = np.concatenate([inp['wq_im'][qperm], inp['wk_im'], inp['wv_im']], 0).T
    # cols: 0-127 q, 128-143 k, 144-159 pad, 160-191 v (32-aligned psum bases)
    Wr = np.concatenate([Wr[:, :144], np.zeros((256, 16), np.float32), Wr[:, 144:]], 1)
    Wi = np.concatenate([Wi[:, :144], np.zeros((256, 16), np.float32), Wi[:, 144:]], 1)
    wstk = np.empty((2, 2, 2, 128, 192), np.float32)  # [outri][cc][ri_in]
    for cc in range(2):
        sl = slice(cc * 128, (cc + 1) * 128)
        wstk[0, cc, 0] = Wr[sl]; wstk[0, cc, 1] = -Wi[sl]
        wstk[1, cc, 0] = Wi[sl]; wstk[1, cc, 1] = Wr[sl]

    # --- eyerow for lam_c fold: [16, 16*128] fp16, eyerow[kk, k*128+c] = (kk==k)
    eyerow = np.zeros((16, 16 * 128), np.float16)
    for k in range(16):
        eyerow[k, k * 128:(k + 1) * 128] = 1.0

    ident = np.eye(128, dtype=np.float32)
    ident16 = np.eye(128, dtype=np.float16)

    # --- BN params tile [128, 8]: q Ar-src cols 0-3 (qs_r qs_i qb_r qb_i),
    # v on rows 0-31 cols 4-7
    bnp = np.zeros((128, 8), np.float32)
    bnp[:, 0] = inp['qs_re'][qperm]; bnp[:, 1] = inp['qs_im'][qperm]
    bnp[:, 2] = inp['qb_re'][qperm]; bnp[:, 3] = inp['qb_im'][qperm]
    bnp[:32, 4] = inp['vs_re']; bnp[:32, 5] = inp['vs_im']
    bnp[:32, 6] = inp['vb_re']; bnp[:32, 7] = inp['vb_im']
    return mk, wstk.reshape(8, 128, 192), eyerow, ident, ident16, bnp


def _build_nc():
    nc = bacc.Bacc("TRN2", target_bir_lowering=False, num_devices=NCORES)
    xf_d = nc.declare_dram_parameter("xf", [8, 128, N2], F32R, isOutput=False)
    w_d = nc.declare_dram_parameter("wstk", [8, 128, 192], F32R, isOutput=False)
    mk_d = nc.declare_dram_parameter("mk", [16, 128, 3840], F16, isOutput=False)
    eye_d = nc.declare_dram_parameter("eyerow", [16, 2048], F16, isOutput=False)
    id_d = nc.declare_dram_parameter("ident", [128, 128], F32, isOutput=False)
    id16_d = nc.declare_dram_parameter("ident16", [128, 128], F16, isOutput=False)
    bnp_d = nc.declare_dram_parameter("bnp", [128, 8], F32, isOutput=False)
    yr_d = nc.declare_dram_parameter("yr", [BL, 64, 4096], F32, isOutput=True)
    yi_d = nc.declare_dram_parameter("yi", [BL, 64, 4096], F32, isOutput=True)
    arin = nc.dram_tensor("arin", [128, 10], F32)
    lam2 = nc.dram_tensor("lam2", [4, 128, 1024], F16)
    qd2 = nc.dram_tensor("qd2", [8, 128, 1024], F16)
    arout = nc.dram_tensor("arout", [128, 10], F32, addr_space="Shared")

    with tile.TileContext(nc) as tc, ExitStack() as ctx:
        per = ctx.enter_context(tc.tile_pool(name="per", bufs=1))   # persistent
        tmp = ctx.enter_context(tc.tile_pool(name="tmp", bufs=2))   # scratch

        wt = [per.tile([128, 192], F32R, tag=f"w{i}", name=f"w{i}") for i in range(8)]
        for i in range(8):
            nc.sync.dma_start(wt[i][:], w_d[i])
        eye = per.tile([16, 2048], F16, tag="eye", name="eye")
        nc.sync.dma_start(eye[:], eye_d[:])
        ident = per.tile([128, 128], F32, tag="ident", name="ident")
        nc.sync.dma_start(ident[:], id_d[:])
        ident16 = per.tile([128, 128], F16, tag="ident16", name="ident16")
        nc.sync.dma_start(ident16[:], id16_d[:])
        bnp = per.tile([128, 8], F32, tag="bnp", name="bnp")
        nc.sync.dma_start(bnp[:], bnp_d[:])

        q_sb = [[per.tile([128, N2], F32, tag=f"q{b}{ri}", name=f"q{b}{ri}") for ri in range(2)]
                for b in range(BL)]
        k_sb = [[per.tile([16, N2], F32, tag=f"k{b}{ri}", name=f"k{b}{ri}") for ri in range(2)]
                for b in range(BL)]
        v_sb = [[per.tile([32, N2], F32, tag=f"v{b}{ri}", name=f"v{b}{ri}") for ri in range(2)]
                for b in range(BL)]

        # ---------------- projections (f32r, N=512) ----------------
        with tc.tile_pool(name="xfp", bufs=1) as xfp, \
             tc.tile_pool(name="pj", bufs=4, space="PSUM") as pj:
            xft = [xfp.tile([128, N2], F32R, tag=f"xf{i}", name=f"xf{i}") for i in range(8)]
            for i in range(8):
                nc.sync.dma_start(xft[i][:], xf_d[i])
            for b in range(BL):
                for ri in range(2):
                    for nch in range(2):
                        pq = pj.tile([128, 512], F32, tag="pq", name="pq")
                        pkv = pj.tile([64, 512], F32, tag="pkv", name="pkv")
                        first = True
                        for cc in range(2):
                            for rin in range(2):
                                lhs = wt[ri * 4 + cc * 2 + rin]
                                rhs = xft[b * 4 + rin * 2 + cc][:, nch * 512:(nch + 1) * 512]
                                nc.tensor.matmul(pq[:], lhs[:, 0:128], rhs,
                                                 start=first, stop=(cc == 1 and rin == 1))
                                nc.tensor.matmul(pkv[:], lhs[:, 128:192], rhs,
                                                 start=first, stop=(cc == 1 and rin == 1))
                                first = False
                        sl = slice(nch * 512, (nch + 1) * 512)
                        nc.vector.tensor_copy(q_sb[b][ri][:, sl], pq[:])
                        nc.vector.tensor_copy(k_sb[b][ri][:, sl], pkv[0:16, :])
                        nc.vector.tensor_copy(v_sb[b][ri][:, sl], pkv[32:64, :])

        # ---------------- BN stats + AllReduce ----------------
        stats = per.tile([128, 10], F32, tag="stats", name="stats")
        st_sc = [tmp.tile([128, 1], F32, tag=f"sc{i}", name=f"sc{i}") for i in range(4)]
        scrap = [tmp.tile([128, N2], F32, tag=f"scr{i}", name=f"scr{i}") for i in range(2)]

        def stat5(planes, rows, coff):
            # planes[b][ri] with `rows` partitions -> stats[:, coff:coff+5]
            for s_i, expr in enumerate(["r", "i", "rr", "ii", "ri"]):
                parts = []
                for b in range(BL):
                    t = st_sc[b]
                    pr, pi = planes[b][0][0:rows, :], planes[b][1][0:rows, :]
                    if expr == "r":
                        nc.vector.tensor_reduce(t[0:rows, :], pr, mybir.AxisListType.X,
                                                mybir.AluOpType.add)
                    elif expr == "i":
                        nc.vector.tensor_reduce(t[0:rows, :], pi, mybir.AxisListType.X,
                                                mybir.AluOpType.add)
                    else:
                        a_, b_ = (pr, pr) if expr == "rr" else (pi, pi) if expr == "ii" else (pr, pi)
                        nc.vector.tensor_mul(scrap[b][0:rows, :], a_, b_)
                        nc.vector.tensor_reduce(t[0:rows, :], scrap[b][0:rows, :],
                                                mybir.AxisListType.X, mybir.AluOpType.add)
                    parts.append(t)
                nc.vector.tensor_add(stats[0:rows, coff + s_i:coff + s_i + 1],
                                     parts[0][0:rows, :], parts[1][0:rows, :])

        stat5(q_sb, 128, 0)
        stat5(v_sb, 32, 5)
        nc.sync.dma_start(arin[:], stats[:])
        nc.gpsimd.collective_compute(
            "AllReduce", mybir.AluOpType.add,
            replica_groups=[list(range(NCORES))],
            ins=[arin[:]], outs=[arout[:]])
        ar = per.tile([128, 10], F32, tag="ar", name="ar")
        nc.sync.dma_start(ar[:], arout[:])

        # ---------------- BN coefficients ----------------
        coef = per.tile([128, 8], F32, tag="coef", name="coef")   # q: Ar Ai Br Bi cols0-3; v cols4-7
        ct = [tmp.tile([128, 1], F32, tag=f"ct{i}", name=f"ct{i}") for i in range(8)]

        def bn_coef(rows, soff, poff, coff):
            r_ = slice(0, rows)
            mr, mi, t0, t1, t2, t3, sr, si = (c[r_, :] for c in ct)
            A = lambda c: ar[r_, soff + c:soff + c + 1]
            P = lambda c: bnp[r_, poff + c:poff + c + 1]
            C = lambda c: coef[r_, coff + c:coff + c + 1]
            inv = 1.0 / NSTAT
            nc.vector.tensor_scalar_mul(mr, A(0), inv)
            nc.vector.tensor_scalar_mul(mi, A(1), inv)
            # zr = (err - eii)/N - mr^2 + mi^2 + EPS
            nc.vector.tensor_sub(t0, A(2), A(3))
            nc.vector.tensor_scalar_mul(t0, t0, inv)
            nc.vector.tensor_mul(t1, mr, mr)
            nc.vector.tensor_sub(t0, t0, t1)
            nc.vector.tensor_mul(t1, mi, mi)
            nc.vector.tensor_add(t0, t0, t1)
            nc.vector.tensor_scalar_add(t0, t0, EPS)          # t0 = zr
            # zi = 2*(eri/N - mr*mi)
            nc.vector.tensor_scalar_mul(t1, A(4), inv)
            nc.vector.tensor_mul(t2, mr, mi)
            nc.vector.tensor_sub(t1, t1, t2)
            nc.vector.tensor_scalar_mul(t1, t1, 2.0)          # t1 = zi
            # mag = sqrt(zr^2+zi^2)
            nc.vector.tensor_mul(t2, t0, t0)
            nc.vector.tensor_mul(t3, t1, t1)
            nc.vector.tensor_add(t2, t2, t3)
            nc.scalar.sqrt(t2, t2)                            # t2 = mag
            # sr = sqrt((mag+zr)/2); si = zi/(2 sr)
            nc.vector.tensor_add(t3, t2, t0)
            nc.scalar.activation(sr, t3, mybir.ActivationFunctionType.Sqrt, scale=0.5)
            nc.vector.reciprocal(t3, sr)
            nc.vector.tensor_mul(si, t1, t3)
            nc.vector.tensor_scalar_mul(si, si, 0.5)          # si = zi/(2 sr)
            nc.vector.reciprocal(t3, t2)                      # t3 = 1/mag
            # Ar = (qsr*sr + qsi*si)/mag ; Ai = (qsi*sr - qsr*si)/mag
            nc.vector.tensor_mul(t0, P(0), sr)
            nc.vector.tensor_mul(t1, P(1), si)
            nc.vector.tensor_add(t0, t0, t1)
            nc.vector.tensor_mul(C(0), t0, t3)
            nc.vector.tensor_mul(t0, P(1), sr)
            nc.vector.tensor_mul(t1, P(0), si)
            nc.vector.tensor_sub(t0, t0, t1)
            nc.vector.tensor_mul(C(1), t0, t3)
            # Br = qbr - Ar*mr + Ai*mi ; Bi = qbi - Ar*mi - Ai*mr
            nc.vector.tensor_mul(t0, C(0), mr)
            nc.vector.tensor_sub(t0, P(2), t0)
            nc.vector.tensor_mul(t1, C(1), mi)
            nc.vector.tensor_add(C(2), t0, t1)
            nc.vector.tensor_mul(t0, C(0), mi)
            nc.vector.tensor_sub(t0, P(3), t0)
            nc.vector.tensor_mul(t1, C(1), mr)
            nc.vector.tensor_sub(C(3), t0, t1)

        bn_coef(128, 0, 0, 0)
        bn_coef(32, 5, 4, 4)

        # ---------------- apply BN -> q16 planes, vbn planes ----------------
        q16 = [[per.tile([128, N2], F16, tag=f"q16{b}{ri}", name=f"q16{b}{ri}") for ri in range(2)]
               for b in range(BL)]
        vbn = [[per.tile([32, N2], F32, tag=f"vbn{b}{ri}", name=f"vbn{b}{ri}") for ri in range(2)]
               for b in range(BL)]

        def bn_apply(src, dst, rows, coff):
            r_ = slice(0, rows)
            C = lambda c: coef[r_, coff + c:coff + c + 1]
            for b in range(BL):
                pr, pi = src[b][0][r_, :], src[b][1][r_, :]
                t0, t1 = scrap[0][r_, :], scrap[1][r_, :]
                nc.vector.tensor_scalar(t0, pr, C(0), C(2),
                                        mybir.AluOpType.mult, mybir.AluOpType.add)
                nc.vector.tensor_scalar_mul(t1, pi, C(1))
                nc.vector.tensor_sub(dst[b][0][r_, :], t0, t1)
                nc.vector.tensor_scalar(t0, pi, C(0), C(3),
                                        mybir.AluOpType.mult, mybir.AluOpType.add)
                nc.vector.tensor_scalar_mul(t1, pr, C(1))
                nc.vector.tensor_add(dst[b][1][r_, :], t0, t1)

        bn_apply(q_sb, q16, 128, 0)
        bn_apply(v_sb, vbn, 32, 4)

        # ---------------- softmax(|k|) + ksmT ----------------
        ksmT = [per.tile([128, 128], F16, tag=f"ksmT{b}", name=f"ksmT{b}") for b in range(BL)]
        qT = [[per.tile([128, 1024], F16, tag=f"qT{b}{ri}", name=f"qT{b}{ri}")
               for ri in range(2)] for b in range(BL)]
        V_rhs = per.tile([128, 1024], F16, tag="vrhs", name="vrhs")
        with tc.tile_pool(name="tp", bufs=2, space="PSUM") as tpp:
            for b in range(BL):
                kr, ki = k_sb[b][0], k_sb[b][1]
                ka = scrap[0][0:16, :]
                t1 = scrap[1][0:16, :]
                nc.vector.tensor_mul(ka, kr, kr)
                nc.vector.tensor_mul(t1, ki, ki)
                nc.vector.tensor_add(ka, ka, t1)
                nc.scalar.sqrt(ka, ka)
                mx = st_sc[0][0:16, :]
                nc.vector.tensor_reduce(mx, ka, mybir.AxisListType.X, mybir.AluOpType.max)
                nc.vector.tensor_scalar(ka, ka, mx, None, mybir.AluOpType.subtract)
                sm = st_sc[1][0:16, :]
                nc.scalar.activation(ka, ka, mybir.ActivationFunctionType.Exp,
                                     accum_out=sm)
                rc = st_sc[2][0:16, :]
                nc.vector.reciprocal(rc, sm)
                nc.vector.tensor_scalar(ka, ka, rc, None, mybir.AluOpType.mult)
                for ch in range(8):
                    pt = tpp.tile([128, 16], F32, tag="pt", name="pt")
                    nc.tensor.transpose(pt[:], ka[:, ch * 128:(ch + 1) * 128],
                                        ident[0:16, 0:16])
                    nc.vector.tensor_copy(ksmT[b][:, ch * 16:(ch + 1) * 16], pt[:])
                for ri in range(2):
                    for nb in range(8):
                        pqz = tpp.tile([128, 128], F16, tag="pqz", name="pqz")
                        nc.tensor.transpose(pqz[:], q16[b][ri][:, nb * 128:(nb + 1) * 128],
                                            ident16[:])
                        nc.vector.tensor_copy(qT[b][ri][:, nb * 128:(nb + 1) * 128], pqz[:])
                # V_rhs[(m),(ri,b,v)] from vbn via PE transpose
                for ri in range(2):
                    for ch in range(8):
                        pv = tpp.tile([128, 32], F32, tag="pv", name="pv")
                        nc.tensor.transpose(pv[:], vbn[b][ri][:, ch * 128:(ch + 1) * 128],
                                            ident[0:32, 0:32])
                        nc.vector.tensor_copy(
                            V_rhs[:, ch * 128 + ri * 64 + b * 32:
                                  ch * 128 + ri * 64 + b * 32 + 32], pv[:])

        # ---------------- lam_c ----------------
        lam_sb = per.tile([16, 128], F16, tag="lamc", name="lamc")
        with tc.tile_pool(name="lc", bufs=2, space="PSUM") as lcp:
            for b in range(BL):
                plc = lcp.tile([16, 64], F32, tag="plc", name="plc")
                for ch in range(8):
                    rhs = V_rhs[:, ch * 128 + b * 32:ch * 128 + b * 32 + 1].copy()
                    rhs.ap = bass_rust.VecI64Pair([(1024, 128), (64, 2), (1, 32)])
                    rhs.offset = ch * 128 + b * 32
                    nc.tensor.matmul(plc[:], ksmT[b][:, ch * 16:(ch + 1) * 16], rhs,
                                     start=(ch == 0), stop=(ch == 7))
                for ri in range(2):
                    nc.vector.tensor_copy(
                        lam_sb[:, ri * 64 + b * 32:ri * 64 + b * 32 + 32],
                        plc[:, ri * 32:(ri + 1) * 32])

        # ---------------- lam_p main loop ----------------
        lam_all = [per.tile([128, 2048], F16, tag=f"lam{nb}", name=f"lam{nb}") for nb in range(8)]
        with tc.tile_pool(name="mk", bufs=2) as mkp, \
             tc.tile_pool(name="p2s", bufs=2) as p2sp, \
             tc.tile_pool(name="lp", bufs=2, space="PSUM") as lpp:
            for k in range(16):
                mkt = mkp.tile([128, 3840], F16, tag="mk", name="mk")
                nc.sync.dma_start(mkt[:], mk_d[k])
                for nb in range(8):
                    P1 = lpp.tile([128, 128], F32, tag="P1", name="P1")
                    P2 = lpp.tile([128, 128], F32, tag="P2", name="P2")
                    nc.tensor.matmul(P1[:], eye[:, k * 128:(k + 1) * 128], lam_sb[:],
                                     start=True, stop=False)
                    for bip in range(8):
                        dx = (bip - nb + 7) * 128
                        rhs = V_rhs[:, bip * 128:(bip + 1) * 128]
                        nc.tensor.matmul(P1[:], mkt[:, dx:dx + 128], rhs,
                                         start=False, stop=(bip == 7))
                        nc.tensor.matmul(P2[:], mkt[:, 1920 + dx:1920 + dx + 128], rhs,
                                         start=(bip == 0), stop=(bip == 7))
                    p2s = p2sp.tile([128, 128], F32, tag="p2s", name="p2s")
                    nc.scalar.copy(p2s[:], P2[:])
                    nc.vector.tensor_sub(lam_all[nb][:, k * 128:k * 128 + 64],
                                         P1[:, 0:64], p2s[:, 64:128])
                    nc.vector.tensor_add(lam_all[nb][:, k * 128 + 64:k * 128 + 128],
                                         P1[:, 64:128], p2s[:, 0:64])

        # ---------------- Lam shuffle + qdiag + Yp ----------------
        qd = [[[per.tile([128, 1024], F16, tag=f"qd{b}{ri}{p}", name=f"qd{b}{ri}{p}") for p in range(2)]
               for ri in range(2)] for b in range(BL)]
        for b in range(BL):
            for ri in range(2):
                for p in range(2):
                    nc.vector.memset(qd[b][ri][p][:], 0.0)
                    nc.sync.dma_start(qd2[(b * 2 + ri) * 2 + p], qd[b][ri][p][:])

        with tc.tile_pool(name="lyp", bufs=2) as lypp, \
             tc.tile_pool(name="yo", bufs=1) as yop, \
             tc.tile_pool(name="yp", bufs=2, space="PSUM") as ypp:
            for b in range(BL):
                y_out = [yop.tile([64, 4096], F32, tag=f"yo{ri}", name=f"yo{ri}")
                         for ri in range(2)]
                for nb in range(8):
                    par = nb % 2
                    lamyp = lypp.tile([128, 1024], F16, tag="lamyp", name="lamyp")
                    lidx = b * 2 + par
                    for t in range(16):
                        for ri in range(2):
                            # SBUF[(t,g),(k,ri,b,v)] -> DRAM[(k,g),(t,ri,v)] scatter
                            sap = lam_all[nb][0:1, 0:1].copy()
                            sap.ap = bass_rust.VecI64Pair(
                                [(2048, 8), (128, 16), (1, 32)])
                            sap.offset = t * 8 * 2048 + ri * 64 + b * 32
                            dap = lam2[0][0:1, 0:1].copy()
                            dap.ap = bass_rust.VecI64Pair(
                                [(1024, 8), (8192, 16), (1, 32)])
                            dap.offset = lidx * 131072 + t * 64 + ri * 32
                            nc.scalar.dma_start(dap, sap)
                            # qT[(t,g),(nb;k,h)] -> DRAM[(k,g),(t,g,h)] scatter
                            qap = qT[b][ri][0:1, 0:1].copy()
                            qap.ap = bass_rust.VecI64Pair(
                                [(1024, 8), (8, 16), (1, 8)])
                            qap.offset = t * 8 * 1024 + nb * 128
                            q2ap = qd2[0][0:1, 0:1].copy()
                            q2ap.ap = bass_rust.VecI64Pair(
                                [(1032, 8), (8192, 16), (1, 8)])
                            q2ap.offset = ((b * 2 + ri) * 2 + par) * 131072 + t * 64
                            nc.sync.dma_start(q2ap, qap)
                    nc.scalar.dma_start(lamyp[:], lam2[lidx])
                    for ri in range(2):
                        nc.sync.dma_start(qd[b][ri][par][:],
                                          qd2[(b * 2 + ri) * 2 + par])
                    for t in range(16):
                        P1y = ypp.tile([64, 64], F32, tag="P1y", name="P1y")
                        P2y = ypp.tile([64, 64], F32, tag="P2y", name="P2y")
                        sl = slice(t * 64, (t + 1) * 64)
                        nc.tensor.matmul(P1y[:], qd[b][0][par][:, sl], lamyp[:, sl],
                                         start=True, stop=True)
                        nc.tensor.matmul(P2y[:], qd[b][1][par][:, sl], lamyp[:, sl],
                                         start=True, stop=True)
                        oc = (nb * 16 + t) * 32
                        p2y = lypp.tile([64, 64], F32, tag="p2y", name="p2y")
                        nc.scalar.copy(p2y[:], P2y[:])
                        nc.vector.tensor_sub(y_out[0][:, oc:oc + 32],
                                             P1y[:, 0:32], p2y[:, 32:64])
                        nc.vector.tensor_add(y_out[1][:, oc:oc + 32],
                                             P1y[:, 32:64], p2y[:, 0:32])
                nc.sync.dma_start(yr_d[b], y_out[0][:])
                nc.sync.dma_start(yi_d[b], y_out[1][:])


    nc.compile()
    return nc


def kernel(**inputs):
    inp = {k: np.asarray(v) for k, v in inputs.items()}
    if "nc" not in _CACHE:
        _CACHE["nc"] = _build_nc()
    nc = _CACHE["nc"]
    mk, wstk, eyerow, ident, ident16, bnp = _build_host_consts(inp)

    xr = inp['x_re'].reshape(B, DIM, N2)
    xi = inp['x_im'].reshape(B, DIM, N2)
    in_maps = []
    for c in range(NCORES):
        xf = np.empty((8, 128, N2), np.float32)
        for bl in range(BL):
            b = c * BL + bl
            for ri, xx in ((0, xr), (1, xi)):
                for cc in range(2):
                    xf[bl * 4 + ri * 2 + cc] = xx[b, cc * 128:(cc + 1) * 128, :]
        in_maps.append({"xf": xf, "wstk": wstk, "mk": mk, "eyerow": eyerow,
                        "ident": ident, "ident16": ident16, "bnp": bnp})
    res = run_bass_kernel_spmd(nc, in_maps, list(range(NCORES)))
    out = np.empty((B, 256, N2), np.complex64)
    for c in range(NCORES):
        yr = res.results[c]["yr"]
        yi = res.results[c]["yi"]
        for bl in range(BL):
            yc = (yr[bl] + 1j * yi[bl]).reshape(8, 8, 128, 32)  # (g,h,nt,v)
            out[c * BL + bl] = yc.transpose(1, 3, 2, 0).reshape(256, N2)
    return out.reshape(B, 256, 32, 32)

